# revision 16
# baseline (speedup 1.0000x reference)
"""FBCritic embedding-lookup kernel for 8 Trainium2 NeuronCores.

Math (reference):
    fwd_idx = clip(obs)*10 + clip(act)            # [8192]
    bwd_idx = clip(fobs)*10 + clip(fact)          # [8192]
    F = W_f[fwd_idx]                              # [8192, 64]
    B = W_b[bwd_idx]                              # [8192, 64]
    out = F @ B.T                                 # [8192, 8192] f32

Sharding (vocab-parallel, per the sharding hint): both tables are sharded
row-wise across the 8 cores in bf16 (125000 rows + a zero pad row each, 32MB
per core vs 4GB replicated f32 — 16x less host->device traffic). Each core
gathers ONLY the rows it owns (owner-compacted indirect DMAs: the host
routes each of the 8192 fwd/bwd positions to the owning core, padded to a
static capacity), scatters them into position-ordered partial buffers
(non-owned rows stay zero), then a ReduceScatter combines the forward
partials (each core receives exactly its 1024-row output block) and an
AllReduce combines the backward partials (every core needs all 8192 backward
rows). Because every position has exactly one owner, the bf16 collective
sums add 7 zeros to 1 real row — numerically exact.

The reduced rows are PE-transposed into [64, n] operands and multiplied in
bf16 [64k x 128m x 512n] tiles into f32 PSUM; PSUM->SBUF copies cast to bf16
(copies alternate vector/scalar engines) and each 128-row strip ships as one
2MB output DMA. The host upcasts to f32 (~0.5% rel err total vs 2e-2 gate).

Indirect DMAs gather/scatter 128 rows each (one offset per partition —
hardware limit; multi-offset APs only work in CoreSim). Owner-compaction
cuts them from 128/core (every core gathers every position) to 40/core at
C=1280, shrinking the dominant gpsimd descriptor-generation chain (~1us
per indirect DMA). C is ~8 sigma above the Binomial(8192, 1/8) ownership
count; if an input ever exceeds it the kernel transparently recompiles
with a larger capacity. Scatter positions are host-permuted (collective
sums are elementwise, so any core-consistent layout works; the F
permutation stays within each 1024-row ReduceScatter chunk) so the
post-collective SBUF loads are single contiguous DMAs. Zero-fills and
loads run on the scalar DMA queue so successive invocations pipeline:
iteration i+1's gather/scatter/collective chain overlaps iteration i's
transpose/matmul/output chain instead of queueing behind its strips.
"""

import numpy as np

NUM_OBS = 100000
NUM_ACT = 10
V = NUM_OBS * NUM_ACT   # 1_000_000 table rows
VS = V // 8             # 125_000 rows per shard
D = 64                  # repr dim
B = 8192                # batch
N_CORES = 8
MR = B // N_CORES       # 1024 output rows per core
P = 128                 # partitions
C_DEFAULT = 1280        # gather capacity per table per core

_CACHE = {}


def _build_nc(C, reps=1):
    """reps>1 repeats the kernel body (same buffers, dependency-chained)
    inside one NEFF — used only for slope-based timing in test.py."""
    import concourse.bass as bass
    import concourse.tile as tile
    from concourse import bacc, mybir

    f32 = mybir.dt.float32
    bf16 = mybir.dt.bfloat16
    i32 = mybir.dt.int32

    G = C // P          # indirect DMAs per table (gather; same count scatter)
    GB = B // P         # 64 backward 128-row blocks
    GF = MR // P        # 8 forward blocks per core

    nc = bacc.Bacc("TRN2", target_bir_lowering=False, debug=False,
                   num_devices=N_CORES)

    wfs = nc.dram_tensor("wfs", [VS + 1, D], bf16, kind="ExternalInput").ap()
    wbs = nc.dram_tensor("wbs", [VS + 1, D], bf16, kind="ExternalInput").ap()
    gidxf_d = nc.dram_tensor("gidxf", [P, G], i32, kind="ExternalInput").ap()
    sposf_d = nc.dram_tensor("sposf", [P, G], i32, kind="ExternalInput").ap()
    gidxb_d = nc.dram_tensor("gidxb", [P, G], i32, kind="ExternalInput").ap()
    sposb_d = nc.dram_tensor("sposb", [P, G], i32, kind="ExternalInput").ap()
    ident_d = nc.dram_tensor("ident", [P, P], bf16, kind="ExternalInput").ap()
    out_d = nc.dram_tensor("out", [MR, B], bf16, kind="ExternalOutput").ap()

    # position-ordered partials; row B is the dump row for capacity padding
    fpart = nc.dram_tensor("fpart", [B + P, D], bf16, kind="Internal").ap()
    bpart = nc.dram_tensor("bpart", [B + P, D], bf16, kind="Internal").ap()
    fred = nc.dram_tensor("fred", [MR, D], bf16, kind="Internal").ap()
    bred = nc.dram_tensor("bred", [B, D], bf16, kind="Internal",
                          addr_space="Shared").ap()

    groups = [list(range(N_CORES))]
    NJ = 512            # matmul moving free dim (one PSUM bank)

    n_copy = [0]

    def alt_copy(dst, src):
        if n_copy[0] % 2 == 0:
            nc.vector.tensor_copy(out=dst, in_=src)
        else:
            nc.scalar.copy(out=dst, in_=src)
        n_copy[0] += 1

    with tile.TileContext(nc) as tc:
        with (
            tc.tile_pool(name="const", bufs=1) as const_pool,
            tc.tile_pool(name="idx", bufs=1) as idx_pool,
            tc.tile_pool(name="gat", bufs=1) as gat_pool,
            tc.tile_pool(name="ld", bufs=1) as ld_pool,
            tc.tile_pool(name="ops", bufs=1) as ops_pool,
            tc.tile_pool(name="strip", bufs=2) as strip_pool,
            tc.tile_pool(name="tpsum", bufs=4, space="PSUM") as tpsum_pool,
            tc.tile_pool(name="mpsum", bufs=4, space="PSUM") as mpsum_pool,
        ):
            identity = const_pool.tile([P, P], bf16)
            nc.scalar.dma_start(identity[:], ident_d[:])

            gidxb = idx_pool.tile([P, G], i32, tag="gidxb")
            sposb = idx_pool.tile([P, G], i32, tag="sposb")
            gidxf = idx_pool.tile([P, G], i32, tag="gidxf")
            sposf = idx_pool.tile([P, G], i32, tag="sposf")
            nc.sync.dma_start(gidxb[:], gidxb_d[:])
            nc.sync.dma_start(sposb[:], sposb_d[:])
            nc.sync.dma_start(gidxf[:], gidxf_d[:])
            nc.sync.dma_start(sposf[:], sposf_d[:])

            # zero-fill source (scatter only writes owned rows)
            z = const_pool.tile([P, (B // P) * D], bf16, tag="zero")
            nc.vector.memset(z[:], 0.0)

            def gather_scatter(table, part, gidx, spos, tag):
                t = gat_pool.tile([P, G * D], bf16, tag=tag)
                for g in range(G):
                    nc.gpsimd.indirect_dma_start(
                        out=t[:, g * D:(g + 1) * D],
                        out_offset=None,
                        in_=table[:],
                        in_offset=bass.IndirectOffsetOnAxis(
                            ap=gidx[:, g:g + 1], axis=0),
                    )
                for g in range(G):
                    nc.gpsimd.indirect_dma_start(
                        out=part[:],
                        out_offset=bass.IndirectOffsetOnAxis(
                            ap=spos[:, g:g + 1], axis=0),
                        in_=t[:, g * D:(g + 1) * D],
                        in_offset=None,
                    )

            def transpose_batch(dstT, src, q, nblk):
                """Transpose nblk [128, 64] blocks into dstT[:, q*512...]."""
                pt = tpsum_pool.tile([D, nblk * P], bf16, tag="pt")
                for r in range(nblk):
                    g = q * 4 + r
                    nc.tensor.transpose(
                        out=pt[:, r * P:(r + 1) * P],
                        in_=src[:, g * D:(g + 1) * D],
                        identity=identity[:],
                    )
                alt_copy(dstT[:, q * 4 * P:q * 4 * P + nblk * P], pt[:])

            for _rep in range(reps):
                # zero the partials (scatter only writes owned rows).
                # scalar queue, NOT sync: on sync they'd sit behind the
                # previous rep's output strips and serialize iterations.
                nc.scalar.dma_start(bpart[0:B, :], z[:])
                nc.scalar.dma_start(fpart[0:B, :], z[:])

                # backward first: its AllReduce gates most of the transposes
                gather_scatter(wbs, bpart, gidxb, sposb, "gb")
                nc.gpsimd.collective_compute(
                    "AllReduce", mybir.AluOpType.add, groups,
                    ins=[bpart[0:B, :]], outs=[bred[:]],
                )
                gather_scatter(wfs, fpart, gidxf, sposf, "gf")
                nc.gpsimd.collective_compute(
                    "ReduceScatter", mybir.AluOpType.add, groups,
                    ins=[fpart[0:B, :]], outs=[fred[:]],
                )

                # load reduced rows. The host permutes scatter positions
                # (collective sums are elementwise, so any consistent
                # layout works) such that these are single contiguous
                # DMAs AND the PE transposes still see natural order:
                # b_in[p, g*64+ch] = bred[p*64+g] = B position g*128+p.
                # scalar queue so they fire on collective completion
                # instead of queueing behind the previous rep's strips.
                b_in = ld_pool.tile([P, GB * D], bf16, tag="b_in")
                nc.scalar.dma_start(b_in[:], bred[:])
                f_in = ld_pool.tile([P, GF * D], bf16, tag="f_in")
                nc.scalar.dma_start(f_in[:], fred[:])

                bwdT = ops_pool.tile([D, B], bf16, tag="bwdT")
                fwdT = ops_pool.tile([D, MR], bf16, tag="fwdT")

                for q in range(GB // 4):
                    transpose_batch(bwdT, b_in, q, 4)
                for q in range(GF // 4):
                    transpose_batch(fwdT, f_in, q, 4)

                for m in range(GF):
                    strip = strip_pool.tile([P, B], bf16, tag="strip")
                    for h in range(B // NJ):
                        ps = mpsum_pool.tile([P, NJ], f32, tag="ps")
                        nc.tensor.matmul(
                            out=ps[:],
                            lhsT=fwdT[:, m * P:(m + 1) * P],
                            rhs=bwdT[:, h * NJ:(h + 1) * NJ],
                            start=True,
                            stop=True,
                        )
                        alt_copy(strip[:, h * NJ:(h + 1) * NJ], ps[:])
                    nc.sync.dma_start(out_d[m * P:(m + 1) * P, :], strip[:])

    nc.compile()
    return nc


def _get_nc(C=C_DEFAULT, reps=1):
    key = ("nc", C, reps)
    if key not in _CACHE:
        _CACHE[key] = _build_nc(C, reps)
    return _CACHE[key]


def _ravel_clip(obs, act):
    o = np.clip(np.asarray(obs).astype(np.int64), 0, NUM_OBS - 1)
    a = np.clip(np.asarray(act).astype(np.int64), 0, NUM_ACT - 1)
    return (o * NUM_ACT + a).astype(np.int32)


def _bf16():
    from concourse import mybir
    return mybir.dt.np(mybir.dt.bfloat16)


def _shards(W, fp_key):
    """Per-core bf16 shards with a trailing zero row, cached by fingerprint."""
    key = ("shard", fp_key)
    if key in _CACHE:
        return _CACHE[key]
    bf16 = _bf16()
    w16 = np.asarray(W, dtype=np.float32).astype(bf16)
    shards = []
    for c in range(N_CORES):
        s = np.empty((VS + 1, D), dtype=bf16)
        s[:VS] = w16[c * VS:(c + 1) * VS]
        s[VS] = 0
        shards.append(s)
    _CACHE[key] = shards
    return shards


def _fingerprint(arr):
    a = np.asarray(arr)
    step = max(1, a.shape[0] // 16)
    return (id(arr), a.shape, a.dtype.str, a[::step, ::8].tobytes())


def _perm_b(r):
    """Partial buffer row for backward position r: row p*64+g holds
    position g*128+p, making the post-AllReduce SBUF load contiguous."""
    return (r % P) * (B // P) + r // P


def _perm_f(r):
    """Row for forward position r: same trick within each 1024-row
    ReduceScatter chunk (chunk boundaries must stay position-ordered)."""
    c, j = r // MR, r % MR
    return c * MR + (j % P) * (MR // P) + j // P


def _route(idx, C, perm):
    """Owner-compacted gather/scatter slot tables, [P, C//P] each, per core.

    Slot [p, g] holds list position g*128+p. Pad slots gather the shard's
    zero row (VS) and scatter to the dump rows (>= B)."""
    gidx_l, spos_l = [], []
    for c in range(N_CORES):
        sel = np.where(idx // VS == c)[0]
        assert len(sel) <= C
        g = np.full(C, VS, np.int32)
        # pad slot k = g*128+p scatters to its partition's dump row B+p
        s = (B + (np.arange(C, dtype=np.int32) % P)).astype(np.int32)
        g[:len(sel)] = idx[sel] - c * VS
        s[:len(sel)] = perm(sel).astype(np.int32)
        gidx_l.append(np.ascontiguousarray(g.reshape(C // P, P).T))
        spos_l.append(np.ascontiguousarray(s.reshape(C // P, P).T))
    return gidx_l, spos_l


def make_in_maps(observations, actions, future_observations, future_actions,
                 W_f, W_b, C=C_DEFAULT):
    fwd_idx = _ravel_clip(observations, actions)
    bwd_idx = _ravel_clip(future_observations, future_actions)
    wf_shards = _shards(W_f, _fingerprint(W_f))
    wb_shards = _shards(W_b, _fingerprint(W_b))
    gidxf, sposf = _route(fwd_idx, C, _perm_f)
    gidxb, sposb = _route(bwd_idx, C, _perm_b)
    ident = np.eye(P, dtype=np.float32).astype(_bf16())
    return [
        {"wfs": wf_shards[c], "wbs": wb_shards[c],
         "gidxf": gidxf[c], "sposf": sposf[c],
         "gidxb": gidxb[c], "sposb": sposb[c], "ident": ident}
        for c in range(N_CORES)
    ]


def assemble_output(results):
    return np.concatenate(
        [results[c]["out"].astype(np.float32) for c in range(N_CORES)], axis=0
    )


def _capacity_for(*idx_arrays):
    need = max(
        int(np.bincount(np.asarray(a) // VS, minlength=N_CORES).max())
        for a in idx_arrays
    )
    C = C_DEFAULT
    while C < need:
        C *= 2
    return C


def kernel(**inputs):
    from concourse.bass_utils import run_bass_kernel_spmd

    fwd_idx = _ravel_clip(inputs["observations"], inputs["actions"])
    bwd_idx = _ravel_clip(inputs["future_observations"],
                          inputs["future_actions"])
    C = _capacity_for(fwd_idx, bwd_idx)
    in_maps = make_in_maps(
        inputs["observations"], inputs["actions"],
        inputs["future_observations"], inputs["future_actions"],
        inputs["W_f"], inputs["W_b"], C=C,
    )
    res = run_bass_kernel_spmd(_get_nc(C), in_maps,
                               core_ids=list(range(N_CORES)))
    return assemble_output(res.results)


# revision 18
# speedup vs baseline: 1.2695x; 1.2695x over previous
"""FBCritic embedding-lookup kernel for 8 Trainium2 NeuronCores.

Math (reference):
    fwd_idx = clip(obs)*10 + clip(act)            # [8192]
    bwd_idx = clip(fobs)*10 + clip(fact)          # [8192]
    F = W_f[fwd_idx]                              # [8192, 64]
    B = W_b[bwd_idx]                              # [8192, 64]
    out = F @ B.T                                 # [8192, 8192] f32

Sharding (vocab-parallel, per the sharding hint): both tables are sharded
row-wise across the 8 cores in bf16 (125000 rows + a zero pad row each, 32MB
per core vs 4GB replicated f32 — 16x less host->device traffic). Each core
gathers ONLY the rows it owns (owner-compacted indirect DMAs: the host
routes each of the 8192 fwd/bwd positions to the owning core, padded to a
static capacity), scatters them into position-ordered partial buffers
(non-owned rows stay zero), then a ReduceScatter combines the forward
partials (each core receives exactly its 1024-row output block) and an
AllReduce combines the backward partials (every core needs all 8192 backward
rows). Because every position has exactly one owner, the bf16 collective
sums add 7 zeros to 1 real row — numerically exact.

The reduced rows are PE-transposed into [64, n] operands and multiplied in
bf16 [64k x 128m x 512n] tiles into f32 PSUM; PSUM->SBUF copies cast to bf16
(copies alternate vector/scalar engines) and each 128-row strip ships as one
2MB output DMA. The host upcasts to f32 (~0.5% rel err total vs 2e-2 gate).

Indirect DMAs gather/scatter 128 rows each (one offset per partition —
hardware limit; multi-offset APs only work in CoreSim). Owner-compaction
cuts them from 128/core (every core gathers every position) to 36/core at
C=1152, shrinking the dominant gpsimd descriptor-generation chain (~1us
per indirect DMA). C is ~4 sigma above the Binomial(8192, 1/8) ownership
count; if an input ever exceeds it the kernel transparently recompiles
with a larger capacity. Scatter positions are host-permuted (collective
sums are elementwise, so any core-consistent layout works; the F
permutation stays within each 1024-row ReduceScatter chunk) so the
post-collective SBUF loads are single contiguous DMAs. Zero-fills and
loads run on the scalar DMA queue so successive invocations pipeline:
iteration i+1's gather/scatter/collective chain overlaps iteration i's
transpose/matmul/output chain instead of queueing behind its strips.
"""

import numpy as np

NUM_OBS = 100000
NUM_ACT = 10
V = NUM_OBS * NUM_ACT   # 1_000_000 table rows
VS = V // 8             # 125_000 rows per shard
D = 64                  # repr dim
B = 8192                # batch
N_CORES = 8
MR = B // N_CORES       # 1024 output rows per core
P = 128                 # partitions
C_DEFAULT = 1152        # gather capacity per table per core

_CACHE = {}


def _build_nc(C, reps=1):
    """reps>1 repeats the kernel body (same buffers, dependency-chained)
    inside one NEFF — used only for slope-based timing in test.py."""
    import concourse.bass as bass
    import concourse.tile as tile
    from concourse import bacc, mybir

    f32 = mybir.dt.float32
    bf16 = mybir.dt.bfloat16
    i32 = mybir.dt.int32

    G = C // P          # indirect DMAs per table (gather; same count scatter)
    GB = B // P         # 64 backward 128-row blocks
    GF = MR // P        # 8 forward blocks per core

    nc = bacc.Bacc("TRN2", target_bir_lowering=False, debug=False,
                   num_devices=N_CORES)

    wfs = nc.dram_tensor("wfs", [VS + 1, D], bf16, kind="ExternalInput").ap()
    wbs = nc.dram_tensor("wbs", [VS + 1, D], bf16, kind="ExternalInput").ap()
    gidxf_d = nc.dram_tensor("gidxf", [P, G], i32, kind="ExternalInput").ap()
    sposf_d = nc.dram_tensor("sposf", [P, G], i32, kind="ExternalInput").ap()
    gidxb_d = nc.dram_tensor("gidxb", [P, G], i32, kind="ExternalInput").ap()
    sposb_d = nc.dram_tensor("sposb", [P, G], i32, kind="ExternalInput").ap()
    ident_d = nc.dram_tensor("ident", [P, P], bf16, kind="ExternalInput").ap()
    out_d = nc.dram_tensor("out", [MR, B], bf16, kind="ExternalOutput").ap()

    # position-ordered partials; row B is the dump row for capacity padding
    fpart = nc.dram_tensor("fpart", [B + P, D], bf16, kind="Internal").ap()
    bpart = nc.dram_tensor("bpart", [B + P, D], bf16, kind="Internal").ap()
    fred = nc.dram_tensor("fred", [MR, D], bf16, kind="Internal").ap()
    bred = nc.dram_tensor("bred", [B, D], bf16, kind="Internal",
                          addr_space="Shared").ap()

    groups = [list(range(N_CORES))]
    NJ = 512            # matmul moving free dim (one PSUM bank)

    n_copy = [0]

    def alt_copy(dst, src):
        if n_copy[0] % 2 == 0:
            nc.vector.tensor_copy(out=dst, in_=src)
        else:
            nc.scalar.copy(out=dst, in_=src)
        n_copy[0] += 1

    with tile.TileContext(nc) as tc:
        with (
            tc.tile_pool(name="const", bufs=1) as const_pool,
            tc.tile_pool(name="idx", bufs=1) as idx_pool,
            tc.tile_pool(name="gat", bufs=1) as gat_pool,
            tc.tile_pool(name="ld", bufs=1) as ld_pool,
            tc.tile_pool(name="ops", bufs=1) as ops_pool,
            tc.tile_pool(name="strip", bufs=2) as strip_pool,
            tc.tile_pool(name="tpsum", bufs=4, space="PSUM") as tpsum_pool,
            tc.tile_pool(name="mpsum", bufs=4, space="PSUM") as mpsum_pool,
        ):
            identity = const_pool.tile([P, P], bf16)
            nc.scalar.dma_start(identity[:], ident_d[:])

            gidxb = idx_pool.tile([P, G], i32, tag="gidxb")
            sposb = idx_pool.tile([P, G], i32, tag="sposb")
            gidxf = idx_pool.tile([P, G], i32, tag="gidxf")
            sposf = idx_pool.tile([P, G], i32, tag="sposf")
            nc.sync.dma_start(gidxb[:], gidxb_d[:])
            nc.sync.dma_start(sposb[:], sposb_d[:])
            nc.sync.dma_start(gidxf[:], gidxf_d[:])
            nc.sync.dma_start(sposf[:], sposf_d[:])

            # zero-fill source (scatter only writes owned rows)
            z = const_pool.tile([P, (B // P) * D], bf16, tag="zero")
            nc.vector.memset(z[:], 0.0)

            def gather_scatter(table, part, gidx, spos, tag):
                t = gat_pool.tile([P, G * D], bf16, tag=tag)
                for g in range(G):
                    nc.gpsimd.indirect_dma_start(
                        out=t[:, g * D:(g + 1) * D],
                        out_offset=None,
                        in_=table[:],
                        in_offset=bass.IndirectOffsetOnAxis(
                            ap=gidx[:, g:g + 1], axis=0),
                    )
                for g in range(G):
                    nc.gpsimd.indirect_dma_start(
                        out=part[:],
                        out_offset=bass.IndirectOffsetOnAxis(
                            ap=spos[:, g:g + 1], axis=0),
                        in_=t[:, g * D:(g + 1) * D],
                        in_offset=None,
                    )

            def transpose_batch(dstT, src, q, nblk):
                """Transpose nblk [128, 64] blocks into dstT[:, q*512...]."""
                pt = tpsum_pool.tile([D, nblk * P], bf16, tag="pt")
                for r in range(nblk):
                    g = q * 4 + r
                    nc.tensor.transpose(
                        out=pt[:, r * P:(r + 1) * P],
                        in_=src[:, g * D:(g + 1) * D],
                        identity=identity[:],
                    )
                alt_copy(dstT[:, q * 4 * P:q * 4 * P + nblk * P], pt[:])

            for _rep in range(reps):
                # zero the partials (scatter only writes owned rows).
                # scalar queue, NOT sync: on sync they'd sit behind the
                # previous rep's output strips and serialize iterations.
                nc.scalar.dma_start(bpart[0:B, :], z[:])
                nc.scalar.dma_start(fpart[0:B, :], z[:])

                # backward first: its AllReduce gates most of the transposes
                gather_scatter(wbs, bpart, gidxb, sposb, "gb")
                nc.gpsimd.collective_compute(
                    "AllReduce", mybir.AluOpType.add, groups,
                    ins=[bpart[0:B, :]], outs=[bred[:]],
                )
                gather_scatter(wfs, fpart, gidxf, sposf, "gf")
                nc.gpsimd.collective_compute(
                    "ReduceScatter", mybir.AluOpType.add, groups,
                    ins=[fpart[0:B, :]], outs=[fred[:]],
                )

                # load reduced rows. The host permutes scatter positions
                # (collective sums are elementwise, so any consistent
                # layout works) such that these are single contiguous
                # DMAs AND the PE transposes still see natural order:
                # b_in[p, g*64+ch] = bred[p*64+g] = B position g*128+p.
                # scalar queue so they fire on collective completion
                # instead of queueing behind the previous rep's strips.
                b_in = ld_pool.tile([P, GB * D], bf16, tag="b_in")
                nc.scalar.dma_start(b_in[:], bred[:])
                f_in = ld_pool.tile([P, GF * D], bf16, tag="f_in")
                nc.scalar.dma_start(f_in[:], fred[:])

                bwdT = ops_pool.tile([D, B], bf16, tag="bwdT")
                fwdT = ops_pool.tile([D, MR], bf16, tag="fwdT")

                for q in range(GB // 4):
                    transpose_batch(bwdT, b_in, q, 4)
                for q in range(GF // 4):
                    transpose_batch(fwdT, f_in, q, 4)

                for m in range(GF):
                    strip = strip_pool.tile([P, B], bf16, tag="strip")
                    for h in range(B // NJ):
                        ps = mpsum_pool.tile([P, NJ], f32, tag="ps")
                        nc.tensor.matmul(
                            out=ps[:],
                            lhsT=fwdT[:, m * P:(m + 1) * P],
                            rhs=bwdT[:, h * NJ:(h + 1) * NJ],
                            start=True,
                            stop=True,
                        )
                        alt_copy(strip[:, h * NJ:(h + 1) * NJ], ps[:])
                    nc.sync.dma_start(out_d[m * P:(m + 1) * P, :], strip[:])

    nc.compile()
    return nc


def _get_nc(C=C_DEFAULT, reps=1):
    key = ("nc", C, reps)
    if key not in _CACHE:
        _CACHE[key] = _build_nc(C, reps)
    return _CACHE[key]


def _ravel_clip(obs, act):
    o = np.clip(np.asarray(obs).astype(np.int64), 0, NUM_OBS - 1)
    a = np.clip(np.asarray(act).astype(np.int64), 0, NUM_ACT - 1)
    return (o * NUM_ACT + a).astype(np.int32)


def _bf16():
    from concourse import mybir
    return mybir.dt.np(mybir.dt.bfloat16)


def _shards(W, fp_key):
    """Per-core bf16 shards with a trailing zero row, cached by fingerprint."""
    key = ("shard", fp_key)
    if key in _CACHE:
        return _CACHE[key]
    bf16 = _bf16()
    w16 = np.asarray(W, dtype=np.float32).astype(bf16)
    shards = []
    for c in range(N_CORES):
        s = np.empty((VS + 1, D), dtype=bf16)
        s[:VS] = w16[c * VS:(c + 1) * VS]
        s[VS] = 0
        shards.append(s)
    _CACHE[key] = shards
    return shards


def _fingerprint(arr):
    a = np.asarray(arr)
    step = max(1, a.shape[0] // 16)
    return (id(arr), a.shape, a.dtype.str, a[::step, ::8].tobytes())


def _perm_b(r):
    """Partial buffer row for backward position r: row p*64+g holds
    position g*128+p, making the post-AllReduce SBUF load contiguous."""
    return (r % P) * (B // P) + r // P


def _perm_f(r):
    """Row for forward position r: same trick within each 1024-row
    ReduceScatter chunk (chunk boundaries must stay position-ordered)."""
    c, j = r // MR, r % MR
    return c * MR + (j % P) * (MR // P) + j // P


def _route(idx, C, perm):
    """Owner-compacted gather/scatter slot tables, [P, C//P] each, per core.

    Slot [p, g] holds list position g*128+p. Pad slots gather the shard's
    zero row (VS) and scatter to the dump rows (>= B)."""
    gidx_l, spos_l = [], []
    for c in range(N_CORES):
        sel = np.where(idx // VS == c)[0]
        assert len(sel) <= C
        g = np.full(C, VS, np.int32)
        # pad slot k = g*128+p scatters to its partition's dump row B+p
        s = (B + (np.arange(C, dtype=np.int32) % P)).astype(np.int32)
        g[:len(sel)] = idx[sel] - c * VS
        s[:len(sel)] = perm(sel).astype(np.int32)
        gidx_l.append(np.ascontiguousarray(g.reshape(C // P, P).T))
        spos_l.append(np.ascontiguousarray(s.reshape(C // P, P).T))
    return gidx_l, spos_l


def make_in_maps(observations, actions, future_observations, future_actions,
                 W_f, W_b, C=C_DEFAULT):
    fwd_idx = _ravel_clip(observations, actions)
    bwd_idx = _ravel_clip(future_observations, future_actions)
    wf_shards = _shards(W_f, _fingerprint(W_f))
    wb_shards = _shards(W_b, _fingerprint(W_b))
    gidxf, sposf = _route(fwd_idx, C, _perm_f)
    gidxb, sposb = _route(bwd_idx, C, _perm_b)
    ident = np.eye(P, dtype=np.float32).astype(_bf16())
    return [
        {"wfs": wf_shards[c], "wbs": wb_shards[c],
         "gidxf": gidxf[c], "sposf": sposf[c],
         "gidxb": gidxb[c], "sposb": sposb[c], "ident": ident}
        for c in range(N_CORES)
    ]


def assemble_output(results):
    return np.concatenate(
        [results[c]["out"].astype(np.float32) for c in range(N_CORES)], axis=0
    )


def _capacity_for(*idx_arrays):
    need = max(
        int(np.bincount(np.asarray(a) // VS, minlength=N_CORES).max())
        for a in idx_arrays
    )
    C = C_DEFAULT
    while C < need:
        C *= 2
    return C


def kernel(**inputs):
    from concourse.bass_utils import run_bass_kernel_spmd

    fwd_idx = _ravel_clip(inputs["observations"], inputs["actions"])
    bwd_idx = _ravel_clip(inputs["future_observations"],
                          inputs["future_actions"])
    C = _capacity_for(fwd_idx, bwd_idx)
    in_maps = make_in_maps(
        inputs["observations"], inputs["actions"],
        inputs["future_observations"], inputs["future_actions"],
        inputs["W_f"], inputs["W_b"], C=C,
    )
    res = run_bass_kernel_spmd(_get_nc(C), in_maps,
                               core_ids=list(range(N_CORES)))
    return assemble_output(res.results)
